# revision 1
# baseline (speedup 1.0000x reference)
"""MoE grouped-GEMM (SiLU-gated FFN) kernel for 8 Trainium2 NeuronCores.

Strategy: expert-parallel with intermediate-dim load balancing.
Each expert's intermediate dim (I=2048) is split into 16 blocks of 128
columns; blocks are grouped into jobs of QB=4 blocks.  The 32 jobs are
LPT-packed onto 8 cores x 4 slots so per-core compute is balanced while
every weight byte is DMA'd exactly once.  Tokens are routed host-side
(free all-to-all); partial down-projection sums across block-jobs are
combined host-side (free reduce).

On-core program (SPMD, identical on all 8 cores):
  phase 1 per i-block: up_T/gate_T [128, ntok] = w.T @ x_T accumulated
  over 8 H-chunks; SiLU (ScalarE); mul + bf16 cast (VectorE).
  phase 2 per token tile: down[128, 1024] accumulated over the job's 4
  i-blocks, written bf16 to a per-slot output buffer.
All matmuls in bf16 with fp32 PSUM accumulation.

Performance notes: this problem sits on the joint roofline ("ridge"):
~17MB/core of DMA (~50us at the ~350GB/s per-core HBM limit) against
~56us of TensorE time; both streams are kept co-resident.  Layouts keep
per-partition contiguous runs >=4KB (HW DMA descriptor efficiency),
input loads are split across the SP and ACT hardware DGE queues in
just-in-time compute order, output stores ride the GpSimd software DGE
(last slot on SP to keep the SWDGE drain off the tail), and the output
is bf16 partition-major ([128, ntiles, H]).  A dummy-matmul bridge at
kernel start opens the PE HAM clock gate (1.2 -> 2.4 GHz needs ~3.4us
of sustained busy) while the first loads land.  Measured ~76-80us on
hardware (run-to-run variance from the free-running HAM window).
"""

import os
import sys
from contextlib import ExitStack

import numpy as np

for _p in ("/opt/trn_rl_repo", "/root/.axon_site/_ro/trn_rl_repo"):
    if os.path.isdir(_p) and _p not in sys.path:
        sys.path.append(_p)

import ml_dtypes  # noqa: E402
import concourse.bass as bass  # noqa: E402
import concourse.mybir as mybir  # noqa: E402
import concourse.tile as tile  # noqa: E402
from concourse import bacc  # noqa: E402
from concourse.bass_utils import run_bass_kernel_spmd  # noqa: E402

BF16 = mybir.dt.bfloat16
F32 = mybir.dt.float32
BF16_NP = ml_dtypes.bfloat16

E, T, H, I = 8, 2048, 1024, 2048
NCORES = 8
TILE = 128
NB = I // TILE  # 16 i-blocks per expert
QB = 4  # i-blocks per job
JOBS_PER_CORE = (E * NB // QB) // NCORES  # 4
HC = H // TILE  # 8 h-chunks


def _schedule(tiles):
    """Pack the 32 (expert, block-chunk) jobs onto 8 cores x 4 slots.

    Returns (cores, slot_shapes): cores[c] = list of JOBS_PER_CORE jobs
    (e, c0) sorted by descending tile count; slot_shapes[s] = token tiles
    allocated to slot s (max over cores), identical for all cores.
    """
    jobs = [(e, c0) for e in range(E) for c0 in range(0, NB, QB)]
    jobs.sort(key=lambda j: -tiles[j[0]])
    cores = [[] for _ in range(NCORES)]
    load = [0] * NCORES
    for j in jobs:
        cands = sorted(
            (c for c in range(NCORES) if len(cores[c]) < JOBS_PER_CORE),
            key=lambda c: (load[c], len(cores[c])),
        )
        c = cands[0]
        cores[c].append(j)
        load[c] += tiles[j[0]]
    for c in range(NCORES):
        cores[c].sort(key=lambda j: -tiles[j[0]])
    slot_shapes = tuple(
        max(tiles[cores[c][s][0]] for c in range(NCORES))
        for s in range(JOBS_PER_CORE)
    )
    # Program slot order: largest slot first (its long phase 1 hides the
    # following slots' weight streams), smallest last for a short tail.
    idx = [s for s in range(JOBS_PER_CORE) if slot_shapes[s] > 0]
    order = sorted(idx, key=lambda s: -slot_shapes[s])
    return cores, slot_shapes, order


def _build(widths):
    """Build the SPMD Bass program for the given exact per-slot token widths."""
    active = [w for w in widths if w > 0]
    ntiles = sum(-(-w // TILE) for w in active)
    ntok = ntiles * TILE
    xcols = HC * sum(active)  # xt free-dim cols (slot-major: [slot][h][tok])

    nc = bacc.Bacc("TRN2", target_bir_lowering=False, debug=False,
                   num_devices=NCORES)
    xt = nc.dram_tensor("xt", [TILE, xcols], BF16, kind="ExternalInput").ap()
    # per-slot concatenated lhsT blocks: [slot][p][b][h][i]
    w1 = nc.dram_tensor("w1", [len(active), TILE, QB, HC, TILE], BF16,
                        kind="ExternalInput").ap()
    w3 = nc.dram_tensor("w3", [len(active), TILE, QB, HC, TILE], BF16,
                        kind="ExternalInput").ap()
    # per-slot w2 rhs blocks: [slot][p(i)][b][n(h)]
    w2 = nc.dram_tensor("w2", [len(active), TILE, QB, H], BF16,
                        kind="ExternalInput").ap()
    # partition-major output: token (tile*128+p) lives at out[p, tile, :]
    out = nc.dram_tensor("out", [TILE, ntiles, H], BF16,
                         kind="ExternalOutput").ap()

    with tile.TileContext(nc) as tc, ExitStack() as ctx:
        xpool = ctx.enter_context(tc.tile_pool(name="x", bufs=3))
        wpool = ctx.enter_context(tc.tile_pool(name="w", bufs=3))
        w2pool = ctx.enter_context(tc.tile_pool(name="w2", bufs=3))
        gpool = ctx.enter_context(tc.tile_pool(name="gated", bufs=2))
        apool = ctx.enter_context(tc.tile_pool(name="act", bufs=3))
        opool = ctx.enter_context(tc.tile_pool(name="osb", bufs=2))
        pup = ctx.enter_context(tc.tile_pool(name="pup", bufs=2, space="PSUM"))
        pgt = ctx.enter_context(tc.tile_pool(name="pgt", bufs=2, space="PSUM"))
        pdn = ctx.enter_context(tc.tile_pool(name="pdn", bufs=3, space="PSUM"))

        # PE warm-up: dummy matmuls on memset tiles while the first loads
        # land, so the HAM clock gate opens (1.2 -> 2.4 GHz) before real
        # work starts.
        wu_pool = ctx.enter_context(tc.tile_pool(name="wu", bufs=1))
        wu_l = wu_pool.tile([TILE, TILE], BF16, tag="wul")
        wu_r = wu_pool.tile([TILE, 512], BF16, tag="wur")
        nc.vector.memset(wu_l[:], 0.0)
        nc.vector.memset(wu_r[:], 0.0)
        wu_ps = pup.tile([TILE, 512], F32, tag="up")
        for _ in range(18):
            nc.tensor.matmul(wu_ps[:], wu_l[:], wu_r[:], start=True, stop=True)

        xoff = 0
        tbase = 0
        for s, N in enumerate(active):
            S = -(-N // TILE)  # token tiles (last may be partial)
            # tokens for this slot, split in two h-halves so phase 1 can
            # start after the first half lands
            xlo = xpool.tile([TILE, HC // 2, N], BF16, tag="xlo")
            xhi = xpool.tile([TILE, HC // 2, N], BF16, tag="xhi")
            w1sb = wpool.tile([TILE, QB, HC, TILE], BF16, tag="w1")
            w3sb = wpool.tile([TILE, QB, HC, TILE], BF16, tag="w3")
            w2sb = w2pool.tile([TILE, QB, H], BF16, tag="w2")
            if s == 0:
                # fine-grained first slot: x-lo + w1 stream on SP queue,
                # x-hi + w3 + w2 on ACT queue, in just-in-time order
                nc.sync.dma_start(xlo[:], xt[:, xoff:xoff + HC * N // 2])
                nc.sync.dma_start(w1sb[:, 0:1], w1[s, :, 0:1])
                nc.scalar.dma_start(xhi[:], xt[:, xoff + HC * N // 2:xoff + HC * N])
                nc.sync.dma_start(w1sb[:, 1:2], w1[s, :, 1:2])
                nc.scalar.dma_start(w3sb[:, 0:1], w3[s, :, 0:1])
                nc.scalar.dma_start(w3sb[:, 1:2], w3[s, :, 1:2])
                nc.sync.dma_start(w1sb[:, 2:QB], w1[s, :, 2:QB])
                nc.scalar.dma_start(w3sb[:, 2:QB], w3[s, :, 2:QB])
            else:
                nc.sync.dma_start(xlo[:], xt[:, xoff:xoff + HC * N // 2])
                nc.sync.dma_start(w1sb[:, 0:QB // 2], w1[s, :, 0:QB // 2])
                nc.sync.dma_start(xhi[:], xt[:, xoff + HC * N // 2:xoff + HC * N])
                nc.sync.dma_start(w1sb[:, QB // 2:QB], w1[s, :, QB // 2:QB])
                nc.scalar.dma_start(w3sb[:, 0:QB // 2], w3[s, :, 0:QB // 2])
                nc.scalar.dma_start(w3sb[:, QB // 2:QB], w3[s, :, QB // 2:QB])
            nc.scalar.dma_start(w2sb[:], w2[s])

            def xs(h, c0, cw):
                half = xlo if h < HC // 2 else xhi
                return half[:, h % (HC // 2), c0:c0 + cw]

            gated = gpool.tile([TILE, QB, N], BF16, tag="gated")
            for b in range(QB):
                for c0 in range(0, N, 512):
                    cw = min(512, N - c0)
                    up = pup.tile([TILE, cw], F32, tag="up")
                    gt = pgt.tile([TILE, cw], F32, tag="gt")
                    for h in range(HC):
                        nc.tensor.matmul(
                            up[:], w1sb[:, b, h, :], xs(h, c0, cw),
                            start=(h == 0), stop=(h == HC - 1))
                    for h in range(HC):
                        nc.tensor.matmul(
                            gt[:], w3sb[:, b, h, :], xs(h, c0, cw),
                            start=(h == 0), stop=(h == HC - 1))
                    act = apool.tile([TILE, cw], F32, tag="act")
                    nc.scalar.activation(act[:], up[:],
                                         mybir.ActivationFunctionType.Silu)
                    nc.vector.tensor_mul(gated[:, b, c0:c0 + cw], act[:], gt[:])

            oslot = opool.tile([TILE, S, H], BF16, tag="osb")
            for t in range(S):
                M = min(TILE, N - t * TILE)  # partial last tile
                for n0 in range(0, H, 512):
                    dn = pdn.tile([M, 512], F32, tag="dn")
                    for b in range(QB):
                        nc.tensor.matmul(
                            dn[:], gated[:, b, t * TILE:t * TILE + M],
                            w2sb[:, b, n0:n0 + 512],
                            start=(b == 0), stop=(b == QB - 1))
                    nc.vector.tensor_copy(oslot[0:M, t, n0:n0 + 512], dn[:])
            if s == len(active) - 1:
                # last slot: SP HW queue is idle by now and avoids putting
                # the GpSimd SWDGE drain on the critical tail
                nc.sync.dma_start(out[:, tbase:tbase + S, :], oslot[:])
            else:
                nc.gpsimd.dma_start(out[:, tbase:tbase + S, :], oslot[:])
            xoff += HC * N
            tbase += S
    nc.compile()
    return nc


def _ensure_ntff_hook():
    """Register the axon NTFF profile hook if the image's antenv lacks it."""
    import types
    try:
        from antenv.axon_hooks import get_axon_ntff_profile_hook  # noqa: F401
        return
    except ImportError:
        pass
    try:
        import antenv
        from trn_agent_boot.trn_boot import _ntff_profile_via_ctypes
        mod = types.ModuleType("antenv.axon_hooks")
        store = [None]
        mod.set_axon_ntff_profile_hook = lambda h: store.__setitem__(0, h)
        mod.get_axon_ntff_profile_hook = lambda: store[0]
        sys.modules["antenv.axon_hooks"] = mod
        antenv.axon_hooks = mod
        inner = _ntff_profile_via_ctypes("/opt/axon/libaxon_pjrt.so")

        import contextlib

        @contextlib.contextmanager
        def hook(output_dir, device_ids):
            # axon_start_nrt_profile needs the PJRT client initialized,
            # which happens on first execute (not on jax.devices()).
            import jax
            import jax.numpy as jnp
            jax.block_until_ready(jnp.add(jnp.ones(8), 1.0))
            with inner(output_dir, device_ids):
                yield

        mod.set_axon_ntff_profile_hook(hook if inner else None)
    except Exception as e:  # profiling is best-effort
        print(f"ntff hook registration failed: {e}", file=sys.stderr)


_CACHE = {}


def _get_program(slot_shapes):
    if slot_shapes not in _CACHE:
        _CACHE[slot_shapes] = _build(slot_shapes)
    return _CACHE[slot_shapes]


def _run(hiddens, w1_weight, w2_weight, w3_weight, batch_sizes, trace=False):
    bs = np.asarray(batch_sizes, dtype=np.int64)
    starts = np.concatenate([[0], np.cumsum(bs)])
    tiles = [int(-(-b // TILE)) for b in bs]
    cores, slot_shapes, order = _schedule(tiles)
    # per-slot token width, rounded to full 128-token tiles: odd widths
    # break DMA run alignment and measure slower than the padding they save
    slot_widths = [slot_shapes[s] * TILE for s in range(JOBS_PER_CORE)]
    widths = tuple(slot_widths[s] for s in order if slot_widths[s] > 0)
    order = [s for s in order if slot_widths[s] > 0]
    ntok = sum(-(-w // TILE) for w in widths) * TILE
    nslot = len(widths)

    nc = _get_program(widths)

    x = np.asarray(hiddens, dtype=np.float32)
    w1f = np.asarray(w1_weight)
    w2f = np.asarray(w2_weight)
    w3f = np.asarray(w3_weight)

    xt_cols = HC * sum(widths)
    in_maps = []
    for c in range(NCORES):
        xt_np = np.zeros((TILE, xt_cols), dtype=BF16_NP)
        w1_np = np.zeros((nslot, TILE, QB, HC, TILE), dtype=BF16_NP)
        w3_np = np.zeros((nslot, TILE, QB, HC, TILE), dtype=BF16_NP)
        w2_np = np.zeros((nslot, TILE, QB, H), dtype=BF16_NP)
        xoff = 0
        si = 0
        for s in order:
            e, c0 = cores[c][s]
            N = int(widths[si])
            n_e = int(bs[e])
            if n_e > 0:
                xe = x[starts[e]:starts[e] + n_e]  # [n_e, H]
                # xt[p, h, t] = xe[t, h*128+p]
                blk = np.zeros((TILE, HC, N), dtype=BF16_NP)
                blk[:, :, :n_e] = (
                    xe.T.reshape(HC, TILE, n_e).transpose(1, 0, 2)
                    .astype(BF16_NP))
                xt_np[:, xoff:xoff + HC * N] = blk.reshape(TILE, HC * N)
            # w1/w3 lhsT: [p(h_in_chunk), b, h_chunk, i]
            w1_np[si] = (
                w1f[e].reshape(HC, TILE, NB, TILE)
                [:, :, c0:c0 + QB, :].transpose(1, 2, 0, 3).astype(BF16_NP))
            w3_np[si] = (
                w3f[e].reshape(HC, TILE, NB, TILE)
                [:, :, c0:c0 + QB, :].transpose(1, 2, 0, 3).astype(BF16_NP))
            # w2 rhs: [p(i_in_block), b, n]
            w2_np[si] = (
                w2f[e].reshape(NB, TILE, H)[c0:c0 + QB]
                .transpose(1, 0, 2).astype(BF16_NP))
            xoff += HC * N
            si += 1
        in_maps.append({"xt": xt_np, "w1": w1_np, "w3": w3_np, "w2": w2_np})

    if trace:
        _ensure_ntff_hook()
    res = run_bass_kernel_spmd(nc, in_maps, core_ids=list(range(NCORES)),
                               trace=trace)

    out_full = np.zeros((T, H), dtype=np.float32)
    for c in range(NCORES):
        # out[p, tile, :] -> token rows (tile*128+p)
        core_out = np.asarray(res.results[c]["out"]).astype(np.float32)
        core_out = core_out.transpose(1, 0, 2).reshape(ntok, H)
        tok0 = 0
        for si, s in enumerate(order):
            e, c0 = cores[c][s]
            n_e = int(bs[e])
            if n_e > 0:
                out_full[starts[e]:starts[e] + n_e] += core_out[tok0:tok0 + n_e]
            tok0 += -(-int(widths[si]) // TILE) * TILE
    return out_full, res


def kernel(hiddens, w1_weight, w2_weight, w3_weight, batch_sizes):
    out, _ = _run(hiddens, w1_weight, w2_weight, w3_weight, batch_sizes)
    return out



# revision 3
# speedup vs baseline: 1.0445x; 1.0445x over previous
"""MoE grouped-GEMM (SiLU-gated FFN) kernel for 8 Trainium2 NeuronCores.

Strategy: expert-parallel with pair-similar-width slots.
Experts are sorted by token count and paired (1st+2nd, 3rd+4th, ...).
Each pair forms one SPMD slot of width W_s = max(pair widths): the pair's
2x16 i-blocks are split into 8 jobs of QB=4 blocks, one per core (cores
0-3 take the bigger expert, 4-7 the smaller).  Every weight byte is
DMA'd exactly once; tokens are routed host-side (free all-to-all); the
4 partial down-projection sums per expert are combined host-side (free
reduce).

On-core program (SPMD, identical on all 8 cores), per slot:
  phase 1 per i-block, per <=512-token chunk: up/gate [128, cw] = w.T @
  x_T accumulated over 8 H-chunks; SiLU (ScalarE); mul + bf16 cast
  (VectorE) -> gated [128, QB, W].
  phase 2 transposed: down.T [128(h), cw] accumulated over the job's 4
  i-blocks (w2 block [128i, 128h] stationary, gated streaming), written
  bf16 to a per-slot output buffer [128, HC, W] -- token dim stays in
  the free dim so no 128-token tile rounding anywhere on the PE.
All matmuls bf16 with fp32 PSUM accumulation.  Slot widths are exact
(computed cols = exact max pair width); DRAM layouts pad widths to 32
(64B-aligned runs).  PSUM: 2 up + 2 gate + 4 down banks = 8.
Inputs are split across the SP and ACT hardware DGE queues with
byte-balance (x-lo + w1 + w2-half | x-hi + w3 + w2-half) in just-in-time
compute order; output stores ride the GpSimd software DGE except the
last slot (SP, idle by then).  A dummy-matmul bridge at kernel start
opens the PE HAM clock gate (1.2 -> 2.4 GHz) while the first loads land.
"""

import os
import sys
from contextlib import ExitStack

import numpy as np

for _p in ("/opt/trn_rl_repo", "/root/.axon_site/_ro/trn_rl_repo"):
    if os.path.isdir(_p) and _p not in sys.path:
        sys.path.append(_p)

import ml_dtypes  # noqa: E402
import concourse.bass as bass  # noqa: E402
import concourse.mybir as mybir  # noqa: E402
import concourse.tile as tile  # noqa: E402
from concourse import bacc  # noqa: E402
from concourse.bass_utils import run_bass_kernel_spmd  # noqa: E402

BF16 = mybir.dt.bfloat16
F32 = mybir.dt.float32
BF16_NP = ml_dtypes.bfloat16

E, T, H, I = 8, 2048, 1024, 2048
NCORES = 8
TILE = 128
NB = I // TILE  # 16 i-blocks per expert
QB = 4  # i-blocks per job
HC = H // TILE  # 8 h-chunks
NWARM = 10  # HAM warm-up dummy matmuls (512 rows each)


def _pad32(w):
    return -(-w // 32) * 32


def _chunks(W):
    """Split width W into balanced chunks of <=512 (PSUM bank limit)."""
    n = max(1, -(-W // 512))
    base = W // n
    rem = W - base * n
    out = []
    c0 = 0
    for i in range(n):
        cw = base + (1 if i < rem else 0)
        out.append((c0, cw))
        c0 += cw
    return out


def _schedule(bs):
    """Pair experts by sorted width.  Returns (slots, widths) where
    slots[s] = (expert_a, expert_b) with N_a >= N_b and widths[s] = N_a,
    sorted by descending width, zero-width slots dropped."""
    order = sorted(range(E), key=lambda e: -bs[e])
    slots = []
    for s in range(E // 2):
        ea, eb = order[2 * s], order[2 * s + 1]
        if bs[ea] > 0:
            slots.append(((ea, eb), int(bs[ea])))
    slots.sort(key=lambda p: -p[1])
    return [p[0] for p in slots], tuple(p[1] for p in slots)


def _build(widths):
    """Build the SPMD Bass program for the given exact slot widths."""
    nslot = len(widths)
    pads = [_pad32(w) for w in widths]
    xcols = HC * sum(pads)

    nc = bacc.Bacc("TRN2", target_bir_lowering=False, debug=False,
                   num_devices=NCORES)
    # x: per-slot [h][tok] blocks, partition = h-in-chunk
    xt = nc.dram_tensor("xt", [TILE, xcols], BF16, kind="ExternalInput").ap()
    # w1/w3 lhsT blocks: [slot][p(h_in_chunk)][b][h_chunk][i]
    w1 = nc.dram_tensor("w1", [nslot, TILE, QB, HC, TILE], BF16,
                        kind="ExternalInput").ap()
    w3 = nc.dram_tensor("w3", [nslot, TILE, QB, HC, TILE], BF16,
                        kind="ExternalInput").ap()
    # w2 lhsT blocks: [slot][p(i_in_block)][b][h_chunk][h_in_chunk]
    w2 = nc.dram_tensor("w2", [nslot, TILE, QB, HC, TILE], BF16,
                        kind="ExternalInput").ap()
    # down.T output: [p(h_in_chunk)][slot-concat of [h_chunk][tok]]
    out = nc.dram_tensor("out", [TILE, HC * sum(pads)], BF16,
                         kind="ExternalOutput").ap()

    with tile.TileContext(nc) as tc, ExitStack() as ctx:
        xpool = ctx.enter_context(tc.tile_pool(name="x", bufs=3))
        wpool = ctx.enter_context(tc.tile_pool(name="w", bufs=3))
        w2pool = ctx.enter_context(tc.tile_pool(name="w2", bufs=3))
        gpool = ctx.enter_context(tc.tile_pool(name="gated", bufs=2))
        apool = ctx.enter_context(tc.tile_pool(name="act", bufs=3))
        opool = ctx.enter_context(tc.tile_pool(name="osb", bufs=2))
        pup = ctx.enter_context(tc.tile_pool(name="pup", bufs=2, space="PSUM"))
        pgt = ctx.enter_context(tc.tile_pool(name="pgt", bufs=2, space="PSUM"))
        pdn = ctx.enter_context(tc.tile_pool(name="pdn", bufs=4, space="PSUM"))

        # PE warm-up: dummy matmuls while the first loads land, so the HAM
        # clock gate opens (1.2 -> 2.4 GHz) before real work starts.
        wu_pool = ctx.enter_context(tc.tile_pool(name="wu", bufs=1))
        wu_l = wu_pool.tile([TILE, TILE], BF16, tag="wul")
        wu_r = wu_pool.tile([TILE, 512], BF16, tag="wur")
        nc.vector.memset(wu_l[:], 0.0)
        nc.vector.memset(wu_r[:], 0.0)
        wu_ps = pup.tile([TILE, 512], F32, tag="up")
        for _ in range(NWARM):
            nc.tensor.matmul(wu_ps[:], wu_l[:], wu_r[:], start=True, stop=True)

        xoff = 0
        for s, W in enumerate(widths):
            P = pads[s]
            ch = _chunks(W)
            # x split in two h-halves so phase 1 starts after half lands
            xlo = xpool.tile([TILE, HC // 2, P], BF16, tag="xlo")
            xhi = xpool.tile([TILE, HC // 2, P], BF16, tag="xhi")
            w1sb = wpool.tile([TILE, QB, HC, TILE], BF16, tag="w1")
            w3sb = wpool.tile([TILE, QB, HC, TILE], BF16, tag="w3")
            w2sb = w2pool.tile([TILE, QB, HC, TILE], BF16, tag="w2")
            half = HC * P // 2
            if s == 0:
                # fine-grained first slot in just-in-time order
                nc.sync.dma_start(xlo[:], xt[:, xoff:xoff + half])
                nc.sync.dma_start(w1sb[:, 0:1], w1[s, :, 0:1])
                nc.scalar.dma_start(xhi[:], xt[:, xoff + half:xoff + 2 * half])
                nc.sync.dma_start(w1sb[:, 1:2], w1[s, :, 1:2])
                nc.scalar.dma_start(w3sb[:, 0:1], w3[s, :, 0:1])
                nc.scalar.dma_start(w3sb[:, 1:2], w3[s, :, 1:2])
                nc.sync.dma_start(w1sb[:, 2:QB], w1[s, :, 2:QB])
                nc.scalar.dma_start(w3sb[:, 2:QB], w3[s, :, 2:QB])
            else:
                nc.sync.dma_start(xlo[:], xt[:, xoff:xoff + half])
                nc.sync.dma_start(w1sb[:, 0:QB // 2], w1[s, :, 0:QB // 2])
                nc.scalar.dma_start(xhi[:], xt[:, xoff + half:xoff + 2 * half])
                nc.sync.dma_start(w1sb[:, QB // 2:QB], w1[s, :, QB // 2:QB])
                nc.scalar.dma_start(w3sb[:, 0:QB // 2], w3[s, :, 0:QB // 2])
                nc.scalar.dma_start(w3sb[:, QB // 2:QB], w3[s, :, QB // 2:QB])
            # w2 split across both queues for byte balance
            nc.sync.dma_start(w2sb[:, 0:QB // 2], w2[s, :, 0:QB // 2])
            nc.scalar.dma_start(w2sb[:, QB // 2:QB], w2[s, :, QB // 2:QB])

            def xs(h, c0, cw):
                hh = xlo if h < HC // 2 else xhi
                return hh[:, h % (HC // 2), c0:c0 + cw]

            # phase 1: gated[i, tok] = silu(w1.T x) * (w3.T x)
            gated = gpool.tile([TILE, QB, W], BF16, tag="gated")
            for b in range(QB):
                for c0, cw in ch:
                    up = pup.tile([TILE, cw], F32, tag="up")
                    gt = pgt.tile([TILE, cw], F32, tag="gt")
                    for h in range(HC):
                        nc.tensor.matmul(
                            up[:], w1sb[:, b, h, :], xs(h, c0, cw),
                            start=(h == 0), stop=(h == HC - 1))
                    for h in range(HC):
                        nc.tensor.matmul(
                            gt[:], w3sb[:, b, h, :], xs(h, c0, cw),
                            start=(h == 0), stop=(h == HC - 1))
                    act = apool.tile([TILE, cw], F32, tag="act")
                    nc.scalar.activation(act[:], up[:],
                                         mybir.ActivationFunctionType.Silu)
                    nc.vector.tensor_mul(gated[:, b, c0:c0 + cw], act[:], gt[:])

            # phase 2 transposed: down.T[h, tok] accumulated over i-blocks
            oslot = opool.tile([TILE, HC, P], BF16, tag="osb")
            cp = 0
            for c0, cw in ch:
                for hc in range(HC):
                    dn = pdn.tile([TILE, cw], F32, tag="dn")
                    for b in range(QB):
                        nc.tensor.matmul(
                            dn[:], w2sb[:, b, hc, :],
                            gated[:, b, c0:c0 + cw],
                            start=(b == 0), stop=(b == QB - 1))
                    # alternate PSUM-drain copies between DVE and ACT
                    if cp % 2 == 0:
                        nc.vector.tensor_copy(oslot[:, hc, c0:c0 + cw], dn[:])
                    else:
                        nc.scalar.copy(oslot[:, hc, c0:c0 + cw], dn[:])
                    cp += 1
            if s == nslot - 1:
                # last slot: SP HW queue is idle by now; keeps the SWDGE
                # drain off the critical tail
                nc.sync.dma_start(out[:, xoff:xoff + HC * P], oslot[:])
            else:
                nc.gpsimd.dma_start(out[:, xoff:xoff + HC * P], oslot[:])
            xoff += HC * P
    nc.compile()
    return nc


def _ensure_ntff_hook():
    """Register the axon NTFF profile hook if the image's antenv lacks it."""
    import types
    try:
        from antenv.axon_hooks import get_axon_ntff_profile_hook  # noqa: F401
        return
    except ImportError:
        pass
    try:
        import antenv
        from trn_agent_boot.trn_boot import _ntff_profile_via_ctypes
        mod = types.ModuleType("antenv.axon_hooks")
        store = [None]
        mod.set_axon_ntff_profile_hook = lambda h: store.__setitem__(0, h)
        mod.get_axon_ntff_profile_hook = lambda: store[0]
        sys.modules["antenv.axon_hooks"] = mod
        antenv.axon_hooks = mod
        inner = _ntff_profile_via_ctypes("/opt/axon/libaxon_pjrt.so")

        import contextlib

        @contextlib.contextmanager
        def hook(output_dir, device_ids):
            import jax
            import jax.numpy as jnp
            jax.block_until_ready(jnp.add(jnp.ones(8), 1.0))
            with inner(output_dir, device_ids):
                yield

        mod.set_axon_ntff_profile_hook(hook if inner else None)
    except Exception as e:  # profiling is best-effort
        print(f"ntff hook registration failed: {e}", file=sys.stderr)


_CACHE = {}


def _get_program(widths):
    if widths not in _CACHE:
        _CACHE[widths] = _build(widths)
    return _CACHE[widths]


def _run(hiddens, w1_weight, w2_weight, w3_weight, batch_sizes, trace=False):
    bs = np.asarray(batch_sizes, dtype=np.int64)
    starts = np.concatenate([[0], np.cumsum(bs)])
    slots, widths = _schedule(bs)
    nslot = len(widths)
    pads = [_pad32(w) for w in widths]

    nc = _get_program(widths)

    x = np.asarray(hiddens, dtype=np.float32)
    w1f = np.asarray(w1_weight)
    w2f = np.asarray(w2_weight)
    w3f = np.asarray(w3_weight)

    xt_cols = HC * sum(pads)
    in_maps = []
    for c in range(NCORES):
        xt_np = np.zeros((TILE, xt_cols), dtype=BF16_NP)
        w1_np = np.zeros((nslot, TILE, QB, HC, TILE), dtype=BF16_NP)
        w3_np = np.zeros((nslot, TILE, QB, HC, TILE), dtype=BF16_NP)
        w2_np = np.zeros((nslot, TILE, QB, HC, TILE), dtype=BF16_NP)
        xoff = 0
        for s in range(nslot):
            e = slots[s][0] if c < 4 else slots[s][1]
            c0 = (c % 4) * QB  # this core's first i-block of the expert
            P = pads[s]
            n_e = int(bs[e])
            if n_e > 0:
                xe = x[starts[e]:starts[e] + n_e]  # [n_e, H]
                blk = np.zeros((TILE, HC, P), dtype=BF16_NP)
                blk[:, :, :n_e] = (
                    xe.T.reshape(HC, TILE, n_e).transpose(1, 0, 2)
                    .astype(BF16_NP))
                xt_np[:, xoff:xoff + HC * P] = blk.reshape(TILE, HC * P)
            # w1/w3 lhsT: [p(h_in_chunk), b, h_chunk, i]
            w1_np[s] = (
                w1f[e].reshape(HC, TILE, NB, TILE)
                [:, :, c0:c0 + QB, :].transpose(1, 2, 0, 3).astype(BF16_NP))
            w3_np[s] = (
                w3f[e].reshape(HC, TILE, NB, TILE)
                [:, :, c0:c0 + QB, :].transpose(1, 2, 0, 3).astype(BF16_NP))
            # w2 lhsT: [p(i_in_block), b, h_chunk, h_in_chunk]
            w2_np[s] = (
                w2f[e].reshape(NB, TILE, HC, TILE)[c0:c0 + QB]
                .transpose(1, 0, 2, 3).astype(BF16_NP))
            xoff += HC * P
        in_maps.append({"xt": xt_np, "w1": w1_np, "w3": w3_np, "w2": w2_np})

    if trace:
        _ensure_ntff_hook()
    res = run_bass_kernel_spmd(nc, in_maps, core_ids=list(range(NCORES)),
                               trace=trace)

    out_full = np.zeros((T, H), dtype=np.float32)
    for c in range(NCORES):
        core_out = np.asarray(res.results[c]["out"]).astype(np.float32)
        xoff = 0
        for s in range(nslot):
            e = slots[s][0] if c < 4 else slots[s][1]
            P = pads[s]
            n_e = int(bs[e])
            if n_e > 0:
                # [128(h_in_chunk), HC, P] -> [n_e, H]
                arr = core_out[:, xoff:xoff + HC * P].reshape(TILE, HC, P)
                part = arr[:, :, :n_e].transpose(2, 1, 0).reshape(n_e, H)
                out_full[starts[e]:starts[e] + n_e] += part
            xoff += HC * P
    return out_full, res


def kernel(hiddens, w1_weight, w2_weight, w3_weight, batch_sizes):
    out, _ = _run(hiddens, w1_weight, w2_weight, w3_weight, batch_sizes)
    return out


# revision 5
# speedup vs baseline: 1.0703x; 1.0247x over previous
"""MoE grouped-GEMM (SiLU-gated FFN) kernel for 8 Trainium2 NeuronCores.

Strategy: expert-parallel with pair-similar-width slots.
Experts are sorted by token count and paired (1st+2nd, 3rd+4th, ...).
Each pair forms one SPMD slot of width W_s = max(pair widths): the pair's
2x16 i-blocks are split into 8 jobs of QB=4 blocks, one per core (cores
0-3 take the bigger expert, 4-7 the smaller).  Every weight byte is
DMA'd exactly once; tokens are routed host-side (free all-to-all); the
4 partial down-projection sums per expert are combined host-side (free
reduce).

On-core program (SPMD, identical on all 8 cores), per slot:
  phase 1 per i-block, per <=512-token chunk: up/gate [128, cw] = w.T @
  x_T accumulated over 8 H-chunks; SiLU (ScalarE); mul + bf16 cast
  (VectorE) -> gated [128, QB, W].
  phase 2 transposed: down.T [128(h), cw] accumulated over the job's 4
  i-blocks (w2 block [128i, 128h] stationary, gated streaming), written
  bf16 to a per-slot output buffer [128, HC, W] -- token dim stays in
  the free dim so no 128-token tile rounding anywhere on the PE.
All matmuls bf16 with fp32 PSUM accumulation.  Slot widths are exact
(computed cols = exact max pair width); DRAM layouts pad chunk widths
to 32 (64B-aligned runs).  PSUM: 2 up + 2 gate + 4 down banks = 8.

Pipeline notes: the framework preamble blocks all engines until ~7us
and a single queue transfer streams at only ~150GB/s, so the input
queues are ordered just-in-time at fine grain: per slot, w1-blk0 ->
x-chunk-lo -> remaining w1 on the SP HW queue, w3-blk0 -> x-chunk-hi ->
remaining w3 on the ACT HW queue, w2 split across both.  Output stores
ride the GpSimd software DGE except the last two slots (ACT / split
SP+ACT, idle by then).  A dummy-matmul bridge at kernel start opens the
PE HAM clock gate (1.2 -> 2.4 GHz) while the first loads land; the
bridge must end exactly when the first real matmul's inputs land or the
HAM re-throttles (idle > ~3.4us) and the whole stream runs half-clock.
"""

import os
import sys
from contextlib import ExitStack

import numpy as np

for _p in ("/opt/trn_rl_repo", "/root/.axon_site/_ro/trn_rl_repo"):
    if os.path.isdir(_p) and _p not in sys.path:
        sys.path.append(_p)

import ml_dtypes  # noqa: E402
import concourse.bass as bass  # noqa: E402
import concourse.mybir as mybir  # noqa: E402
import concourse.tile as tile  # noqa: E402
from concourse import bacc  # noqa: E402
from concourse.bass_utils import run_bass_kernel_spmd  # noqa: E402

BF16 = mybir.dt.bfloat16
F32 = mybir.dt.float32
BF16_NP = ml_dtypes.bfloat16

E, T, H, I = 8, 2048, 1024, 2048
NCORES = 8
TILE = 128
NB = I // TILE  # 16 i-blocks per expert
QB = 4  # i-blocks per job
HC = H // TILE  # 8 h-chunks
NWARM = 10  # HAM warm-up dummy matmuls (512 rows each)


def _pad32(w):
    return -(-w // 32) * 32


def _chunks(W):
    """Split width W into balanced chunks of <=512 (PSUM bank limit)."""
    n = max(1, -(-W // 512))
    base = W // n
    rem = W - base * n
    out = []
    c0 = 0
    for i in range(n):
        cw = base + (1 if i < rem else 0)
        out.append((c0, cw))
        c0 += cw
    return out


def _schedule(bs):
    """Pair experts by sorted width.  Returns (slots, widths) where
    slots[s] = (expert_a, expert_b) with N_a >= N_b and widths[s] = N_a,
    sorted by descending width, zero-width slots dropped."""
    order = sorted(range(E), key=lambda e: -bs[e])
    slots = []
    for s in range(E // 2):
        ea, eb = order[2 * s], order[2 * s + 1]
        if bs[ea] > 0:
            slots.append(((ea, eb), int(bs[ea])))
    slots.sort(key=lambda p: -p[1])
    return [p[0] for p in slots], tuple(p[1] for p in slots)


def _xcols(widths):
    """Total xt free-dim cols: per slot, per chunk, lo+hi h-halves of
    padded chunk width."""
    tot = 0
    for W in widths:
        for _, cw in _chunks(W):
            tot += HC * _pad32(cw)
    return tot


def _build(widths):
    """Build the SPMD Bass program for the given exact slot widths."""
    nslot = len(widths)
    pads = [_pad32(w) for w in widths]

    nc = bacc.Bacc("TRN2", target_bir_lowering=False, debug=False,
                   num_devices=NCORES)
    # x: per-slot, per-chunk [h][tok] blocks (lo half then hi half)
    xt = nc.dram_tensor("xt", [TILE, _xcols(widths)], BF16,
                        kind="ExternalInput").ap()
    # w1/w3 lhsT blocks: [slot][p(h_in_chunk)][b][h_chunk][i]
    w1 = nc.dram_tensor("w1", [nslot, TILE, QB, HC, TILE], BF16,
                        kind="ExternalInput").ap()
    w3 = nc.dram_tensor("w3", [nslot, TILE, QB, HC, TILE], BF16,
                        kind="ExternalInput").ap()
    # w2 lhsT blocks: [slot][p(i_in_block)][b][h_chunk][h_in_chunk]
    w2 = nc.dram_tensor("w2", [nslot, TILE, QB, HC, TILE], BF16,
                        kind="ExternalInput").ap()
    # down.T output: [p(h_in_chunk)][slot-concat of [h_chunk][tok]]
    out = nc.dram_tensor("out", [TILE, HC * sum(pads)], BF16,
                         kind="ExternalOutput").ap()

    with tile.TileContext(nc) as tc, ExitStack() as ctx:
        xpool = ctx.enter_context(tc.tile_pool(name="x", bufs=3))
        wpool = ctx.enter_context(tc.tile_pool(name="w", bufs=3))
        w2pool = ctx.enter_context(tc.tile_pool(name="w2", bufs=3))
        gpool = ctx.enter_context(tc.tile_pool(name="gated", bufs=2))
        apool = ctx.enter_context(tc.tile_pool(name="act", bufs=3))
        opool = ctx.enter_context(tc.tile_pool(name="osb", bufs=2))
        pup = ctx.enter_context(tc.tile_pool(name="pup", bufs=2, space="PSUM"))
        pgt = ctx.enter_context(tc.tile_pool(name="pgt", bufs=2, space="PSUM"))
        pdn = ctx.enter_context(tc.tile_pool(name="pdn", bufs=4, space="PSUM"))

        # PE warm-up: dummy matmuls while the first loads land, so the HAM
        # clock gate opens (1.2 -> 2.4 GHz) before real work starts.
        wu_pool = ctx.enter_context(tc.tile_pool(name="wu", bufs=1))
        wu_l = wu_pool.tile([TILE, TILE], BF16, tag="wul")
        wu_r = wu_pool.tile([TILE, 512], BF16, tag="wur")
        nc.vector.memset(wu_l[:], 0.0)
        nc.vector.memset(wu_r[:], 0.0)
        wu_ps = pup.tile([TILE, 512], F32, tag="up")
        for _ in range(NWARM):
            nc.tensor.matmul(wu_ps[:], wu_l[:], wu_r[:], start=True, stop=True)

        xoff = 0
        for s, W in enumerate(widths):
            P = pads[s]
            ch = _chunks(W)
            # per-chunk x tiles, split in lo/hi h-halves (separate DMAs on
            # the two HW queues so the first matmul's inputs land early)
            xlo, xhi = [], []
            w1sb = wpool.tile([TILE, QB, HC, TILE], BF16, tag="w1")
            w3sb = wpool.tile([TILE, QB, HC, TILE], BF16, tag="w3")
            w2sb = w2pool.tile([TILE, QB, HC, TILE], BF16, tag="w2")
            # just-in-time interleave: x chunk 0 first (in lo/hi halves on
            # the two queues), then w blk0, then remaining chunks/blocks
            for k, (c0, cw) in enumerate(ch):
                cp = _pad32(cw)
                half = HC * cp // 2
                lo = xpool.tile([TILE, HC // 2, cp], BF16, tag=f"xlo{k}")
                hi = xpool.tile([TILE, HC // 2, cp], BF16, tag=f"xhi{k}")
                nc.sync.dma_start(lo[:], xt[:, xoff:xoff + half])
                nc.scalar.dma_start(hi[:], xt[:, xoff + half:xoff + 2 * half])
                xlo.append(lo)
                xhi.append(hi)
                xoff += 2 * half
                if k == 0:
                    nc.sync.dma_start(w1sb[:, 0:1], w1[s, :, 0:1])
                    nc.scalar.dma_start(w3sb[:, 0:1], w3[s, :, 0:1])
            nc.sync.dma_start(w1sb[:, 1:2], w1[s, :, 1:2])
            nc.scalar.dma_start(w3sb[:, 1:2], w3[s, :, 1:2])
            nc.sync.dma_start(w1sb[:, 2:QB], w1[s, :, 2:QB])
            nc.scalar.dma_start(w3sb[:, 2:QB], w3[s, :, 2:QB])
            # w2 split across both queues for byte balance
            nc.sync.dma_start(w2sb[:, 0:QB // 2], w2[s, :, 0:QB // 2])
            nc.scalar.dma_start(w2sb[:, QB // 2:QB], w2[s, :, QB // 2:QB])

            def xs(h, k, cw):
                hh = xlo[k] if h < HC // 2 else xhi[k]
                return hh[:, h % (HC // 2), 0:cw]

            # phase 1: gated[i, tok] = silu(w1.T x) * (w3.T x)
            gated = gpool.tile([TILE, QB, W], BF16, tag="gated")
            for b in range(QB):
                for k, (c0, cw) in enumerate(ch):
                    up = pup.tile([TILE, cw], F32, tag="up")
                    gt = pgt.tile([TILE, cw], F32, tag="gt")
                    for h in range(HC):
                        nc.tensor.matmul(
                            up[:], w1sb[:, b, h, :], xs(h, k, cw),
                            start=(h == 0), stop=(h == HC - 1))
                    for h in range(HC):
                        nc.tensor.matmul(
                            gt[:], w3sb[:, b, h, :], xs(h, k, cw),
                            start=(h == 0), stop=(h == HC - 1))
                    act = apool.tile([TILE, cw], F32, tag="act")
                    nc.scalar.activation(act[:], up[:],
                                         mybir.ActivationFunctionType.Silu)
                    nc.vector.tensor_mul(gated[:, b, c0:c0 + cw], act[:], gt[:])

            # phase 2 transposed: down.T[h, tok] accumulated over i-blocks
            oslot = opool.tile([TILE, HC, P], BF16, tag="osb")
            cp = 0
            for c0, cw in ch:
                for hc in range(HC):
                    dn = pdn.tile([TILE, cw], F32, tag="dn")
                    for b in range(QB):
                        nc.tensor.matmul(
                            dn[:], w2sb[:, b, hc, :],
                            gated[:, b, c0:c0 + cw],
                            start=(b == 0), stop=(b == QB - 1))
                    # alternate PSUM-drain copies between DVE and ACT
                    if cp % 2 == 0:
                        nc.vector.tensor_copy(oslot[:, hc, c0:c0 + cw], dn[:])
                    else:
                        nc.scalar.copy(oslot[:, hc, c0:c0 + cw], dn[:])
                    cp += 1
            obase = HC * sum(pads[:s])
            if s == nslot - 1:
                # last slot: HW queues idle by now; split halves for speed
                nc.sync.dma_start(out[:, obase:obase + HC * P // 2],
                                  oslot[:, 0:HC // 2, :])
                nc.scalar.dma_start(out[:, obase + HC * P // 2:obase + HC * P],
                                    oslot[:, HC // 2:HC, :])
            elif s == nslot - 2:
                nc.scalar.dma_start(out[:, obase:obase + HC * P], oslot[:])
            else:
                nc.gpsimd.dma_start(out[:, obase:obase + HC * P], oslot[:])
    nc.compile()
    return nc


def _ensure_ntff_hook():
    """Register the axon NTFF profile hook if the image's antenv lacks it."""
    import types
    try:
        from antenv.axon_hooks import get_axon_ntff_profile_hook  # noqa: F401
        return
    except ImportError:
        pass
    try:
        import antenv
        from trn_agent_boot.trn_boot import _ntff_profile_via_ctypes
        mod = types.ModuleType("antenv.axon_hooks")
        store = [None]
        mod.set_axon_ntff_profile_hook = lambda h: store.__setitem__(0, h)
        mod.get_axon_ntff_profile_hook = lambda: store[0]
        sys.modules["antenv.axon_hooks"] = mod
        antenv.axon_hooks = mod
        inner = _ntff_profile_via_ctypes("/opt/axon/libaxon_pjrt.so")

        import contextlib

        @contextlib.contextmanager
        def hook(output_dir, device_ids):
            import jax
            import jax.numpy as jnp
            jax.block_until_ready(jnp.add(jnp.ones(8), 1.0))
            with inner(output_dir, device_ids):
                yield

        mod.set_axon_ntff_profile_hook(hook if inner else None)
    except Exception as e:  # profiling is best-effort
        print(f"ntff hook registration failed: {e}", file=sys.stderr)


_CACHE = {}


def _get_program(widths):
    if widths not in _CACHE:
        _CACHE[widths] = _build(widths)
    return _CACHE[widths]


def _run(hiddens, w1_weight, w2_weight, w3_weight, batch_sizes, trace=False):
    bs = np.asarray(batch_sizes, dtype=np.int64)
    starts = np.concatenate([[0], np.cumsum(bs)])
    slots, widths = _schedule(bs)
    nslot = len(widths)
    pads = [_pad32(w) for w in widths]

    nc = _get_program(widths)

    x = np.asarray(hiddens, dtype=np.float32)
    w1f = np.asarray(w1_weight)
    w2f = np.asarray(w2_weight)
    w3f = np.asarray(w3_weight)

    xt_cols = _xcols(widths)
    in_maps = []
    for c in range(NCORES):
        xt_np = np.zeros((TILE, xt_cols), dtype=BF16_NP)
        w1_np = np.zeros((nslot, TILE, QB, HC, TILE), dtype=BF16_NP)
        w3_np = np.zeros((nslot, TILE, QB, HC, TILE), dtype=BF16_NP)
        w2_np = np.zeros((nslot, TILE, QB, HC, TILE), dtype=BF16_NP)
        xoff = 0
        for s in range(nslot):
            e = slots[s][0] if c < 4 else slots[s][1]
            c0b = (c % 4) * QB  # this core's first i-block of the expert
            n_e = int(bs[e])
            xe = None
            if n_e > 0:
                xe = x[starts[e]:starts[e] + n_e]  # [n_e, H]
                # xeT[p, h, t] = xe[t, h*128+p]
                xeT = np.ascontiguousarray(
                    xe.T.reshape(HC, TILE, n_e).transpose(1, 0, 2)
                ).astype(BF16_NP)
            for c0, cw in _chunks(widths[s]):
                cp = _pad32(cw)
                if xe is not None and c0 < n_e:
                    m = min(cw, n_e - c0)
                    blk = np.zeros((TILE, HC, cp), dtype=BF16_NP)
                    blk[:, :, :m] = xeT[:, :, c0:c0 + m]
                    xt_np[:, xoff:xoff + HC * cp] = blk.reshape(TILE, HC * cp)
                xoff += HC * cp
            # w1/w3 lhsT: [p(h_in_chunk), b, h_chunk, i]
            w1_np[s] = (
                w1f[e].reshape(HC, TILE, NB, TILE)
                [:, :, c0b:c0b + QB, :].transpose(1, 2, 0, 3).astype(BF16_NP))
            w3_np[s] = (
                w3f[e].reshape(HC, TILE, NB, TILE)
                [:, :, c0b:c0b + QB, :].transpose(1, 2, 0, 3).astype(BF16_NP))
            # w2 lhsT: [p(i_in_block), b, h_chunk, h_in_chunk]
            w2_np[s] = (
                w2f[e].reshape(NB, TILE, HC, TILE)[c0b:c0b + QB]
                .transpose(1, 0, 2, 3).astype(BF16_NP))
        in_maps.append({"xt": xt_np, "w1": w1_np, "w3": w3_np, "w2": w2_np})

    if trace:
        _ensure_ntff_hook()
    res = run_bass_kernel_spmd(nc, in_maps, core_ids=list(range(NCORES)),
                               trace=trace)

    out_full = np.zeros((T, H), dtype=np.float32)
    for c in range(NCORES):
        core_out = np.asarray(res.results[c]["out"]).astype(np.float32)
        xoff = 0
        for s in range(nslot):
            e = slots[s][0] if c < 4 else slots[s][1]
            P = pads[s]
            n_e = int(bs[e])
            if n_e > 0:
                # [128(h_in_chunk), HC, P] -> [n_e, H]
                arr = core_out[:, xoff:xoff + HC * P].reshape(TILE, HC, P)
                part = arr[:, :, :n_e].transpose(2, 1, 0).reshape(n_e, H)
                out_full[starts[e]:starts[e] + n_e] += part
            xoff += HC * P
        assert xoff == core_out.shape[1]
    return out_full, res


def kernel(hiddens, w1_weight, w2_weight, w3_weight, batch_sizes):
    out, _ = _run(hiddens, w1_weight, w2_weight, w3_weight, batch_sizes)
    return out
